# revision 14
# baseline (speedup 1.0000x reference)
"""Trainium2 Bass kernel for nn_CausalAttention (gated-resnet q/k/v projections
+ causal attention). Data-parallel over batch: 8 batches -> 8 NeuronCores.

Per-core computation (batch b):
  x_q = query[b] (C=256, S=1024)   x_k = key[b] (256, 1024)
  branch(p, x): e  = elu(x)
                h1 = W1 @ e + b1 ; e1 = elu(h1)
                h2 = W2 @ e1 + b2 ; a, g = split(h2)
                gr = x + a * sigmoid(g)
                o  = Wn @ gr + bn          (512, 1024) channel-major
  q = branch(q, x_q); k = branch(k, x_k); v = branch(k-input, x_k)
  att view: X_att[s, d] = X_cm[s//2, (s%2)*512 + d]  (flat reinterpretation)
  per head n (d = 64n..64n+63):
    scoresT[s2, s1] = sum_d K_att[s2,d] Q_att[s1,d]   (s2 causal blocks)
    eT = exp(scoresT/sqrt(512)) with strict-lower mask (s2 < s1)
    outT[vs, s1] = sum_s2 V_att[s2, 64n+vs] * eT[s2, s1] ; l[s1] = sum_s2 eT
    final[64n+vs, s1] = outT[vs, s1] / l[s1]   (l[0] patched to 1)

Implementation notes:
  - elu computed in shifted form elus = elu(x)+1 = relu(x) + min(exp(x),1);
    the -1 is folded into the next layer's bias host-side
    (b_adj = b - W.sum(axis=1)).
  - engines: exp/tanh/identity on ACT (one table), relu/ha on Pool (gpsimd),
    combines on DVE (bf16 2x), masks on DVE, ul copies on Pool.
  - branch stages emitted interleaved (v,k,q) so PE never starves.
"""

import os
import sys
import numpy as np

sys.path.insert(0, "/opt/trn_rl_repo")

C = 256
S = 1024
D = 512
NH = 8
KS = 64
VS = 64
SCALE = 1.0 / float(np.sqrt(512.0))
N_CORES = 8

# gpsimd elementwise measured ~14 cyc/elem + SBUF port contention with DVE:
# keep Pool for memset/affine_select/DMA issue only.
CFG = {
    "mask_engine": "vector",     # eT diag mask TT mult
    "ul_engine": "vector",       # psum->sbuf unnorm copy (psum: DVE/ACT only)
    "nin_bias_engine": "vector", # qT/kT bias add (psum: DVE/ACT only)
    "gr_engine": "vector",       # gr = u + x
    "fin_engine": "vector",      # fin = ul * rbc
    "h1relu": "scalar",          # relu(h1+b1) via ACT Relu | vector TS
}


def _split_psum_ranges(a, b, max_n=512):
    """Split [a, b) psum column range into chunks that don't cross 512-col
    bank boundaries and are <= max_n wide."""
    out = []
    while a < b:
        nxt = min(b, ((a // 512) + 1) * 512, a + max_n)
        out.append((a, nxt))
        a = nxt
    return out


def build_program(cfg=CFG):
    from contextlib import ExitStack

    import concourse.bacc as bacc
    import concourse.bass as bass
    import concourse.tile as tile
    from concourse import mybir
    from concourse.alu_op_type import AluOpType as Op

    f32 = mybir.dt.float32
    bf16 = mybir.dt.bfloat16
    AF = mybir.ActivationFunctionType

    nc = bacc.Bacc("TRN2", target_bir_lowering=False, debug=False,
                   num_devices=N_CORES)

    # ---------------- DRAM parameters ----------------
    query = nc.dram_tensor("query", [C, S], bf16, kind="ExternalInput").ap()
    key = nc.dram_tensor("key", [C, S], bf16, kind="ExternalInput").ap()
    # all weights packed [C, 3*1280]: per branch (v,k,q): w1T(256)|w2T(512)|wnT(512)
    wall = nc.dram_tensor("wall", [C, 3 * 1280], bf16, kind="ExternalInput").ap()
    # per-partition biases [128, 18]: per branch: b1_adj(2)|b2a_adj(2)|b2gh_adj(2)
    ball = nc.dram_tensor("ball", [128, 18], f32, kind="ExternalInput").ap()
    # v nin bias per-partition [128, 4]
    bnv_d = nc.dram_tensor("bnv", [128, 4], f32, kind="ExternalInput").ap()
    # q|k nin biases, interleaved [slot, 2a+jj] = bn[slot][a], for broadcast
    bnqk = nc.dram_tensor("bnqk", [2 * 2 * D], f32, kind="ExternalInput").ap()
    out_d = nc.dram_tensor("out", [D, S], f32, kind="ExternalOutput").ap()

    BR = {"v": 0, "k": 1, "q": 2}  # branch order / wall offsets

    def eng(name):
        return getattr(nc, name)

    with tile.TileContext(nc) as tc, ExitStack() as ctx:
        persist = ctx.enter_context(tc.tile_pool(name="persist", bufs=1))
        ctx_br = ExitStack()
        psum_main = ctx_br.enter_context(tc.tile_pool(name="psum_br", bufs=4, space="PSUM"))
        dram_pool = ctx.enter_context(tc.tile_pool(name="dram", bufs=1, space="DRAM"))

        # persistent tiles
        xq = persist.tile([128, 2, S], bf16)
        xk = persist.tile([128, 2, S], bf16)
        eluq = persist.tile([128, 2, S], bf16)   # elu(x)+1
        eluk = persist.tile([128, 2, S], bf16)
        w_all = persist.tile([128, 2, 3 * 1280], bf16)   # [cin%128, cin//128, col]
        bb = persist.tile([128, 18], f32)
        bnv = persist.tile([128, 4], f32)
        bnb = persist.tile([128, 2, 2 * D], f32)  # broadcast q|k nin bias (interleaved)
        qT_m = persist.tile([128, 4, S], bf16)   # Q^T_att: [dd%128, dd//128, s]
        kT_m = persist.tile([128, 4, S], bf16)
        v_aug = persist.tile([128, 8, NH, VS + 1], bf16)  # [s%128, s//128, n, vs|1]
        mask01 = persist.tile([128, 128], bf16)  # [t2, t1] = 1.0 if t1 > t2 else 0
        # l and 1/l packed [16 rows x 64] per head; pair g owns partitions
        # 32g..32g+31 so the reciprocal's start partition is 32-aligned and its
        # free size is only 64 (reciprocal cost scales with free size)
        lbuf = persist.tile([128, 64], f32)
        rbuf = persist.tile([128, 64], f32)

        vproj_dram = dram_pool.tile([D, S], bf16)
        rbuf_dram = dram_pool.tile([128, 64], f32)

        # PE warm-up (p-state ramp): back-to-back matmuls on scratch data
        warm = persist.tile([128, 512], bf16, name="warm")
        nc.vector.memset(warm, 0.5)
        wps = psum_main.tile([128, 1024], f32, tag="pm", name="wps")
        for _ in range(18):
            nc.tensor.matmul(wps[:, 0:512], lhsT=warm[:, 0:128],
                             rhs=warm, start=True, stop=True)

        # ---------------- input + weight DMAs ----------------
        nc.sync.dma_start(out=xq, in_=query.rearrange("(cc p) s -> p cc s", p=128))
        nc.sync.dma_start(out=xk, in_=key.rearrange("(cc p) s -> p cc s", p=128))
        wall4 = wall.rearrange("(kc p) c -> p kc c", p=128)
        for b in range(3):
            nc.sync.dma_start(out=w_all[:, :, b * 1280:(b + 1) * 1280],
                              in_=wall4[:, :, b * 1280:(b + 1) * 1280])
        nc.sync.dma_start(out=bb, in_=ball)
        nc.sync.dma_start(out=bnv, in_=bnv_d)
        bn_bcast = bass.AP(tensor=bnqk.tensor, offset=bnqk.offset,
                           ap=[[0, 128]] + list(bnqk.ap))
        nc.sync.dma_start(out=bnb.rearrange("p a b -> p (a b)"), in_=bn_bcast)

        # mask + v_aug ones columns
        nc.gpsimd.memset(mask01, 1.0)
        nc.gpsimd.affine_select(
            out=mask01, in_=mask01, compare_op=Op.is_ge, fill=0.0,
            base=-1, pattern=[[1, 128]], channel_multiplier=-1)
        nc.gpsimd.memset(v_aug[:, :, :, VS:VS + 1], 1.0)

        # bias column helpers: bb cols per branch b0=6*BR[p]:
        #   b1_adj: b0+0,b0+1 ; b2a_adj: b0+2,b0+3 ; b2gh_adj: b0+4,b0+5
        def bcol(p, kind, i):
            b0 = 6 * BR[p]
            off = {"b1": 0, "b2a": 2, "b2gh": 4}[kind]
            return bb[:, b0 + off + i:b0 + off + i + 1]

        work = ctx.enter_context(tc.tile_pool(name="work", bufs=8))
        e1_pool = ctx.enter_context(tc.tile_pool(name="e1", bufs=2))
        gr_pool = ctx.enter_context(tc.tile_pool(name="gr", bufs=3))

        # ---------------- input elus (shifted): elus = relu(x)+min(exp(x),1)
        def elu_in(x3, dst3):
            x2 = x3.rearrange("p a b -> p (a b)")
            d2 = dst3.rearrange("p a b -> p (a b)")
            r = work.tile([128, 2 * S], bf16, tag="wk2", bufs=4)
            e = work.tile([128, 2 * S], bf16, tag="wk2", bufs=4)
            nc.scalar.activation(r, x2, AF.Relu)
            nc.scalar.activation(e, x2, AF.Exp)
            nc.vector.scalar_tensor_tensor(d2, e, 1.0, r, Op.min, Op.add)

        elu_in(xk, eluk)
        elu_in(xq, eluq)

        # ---------------- branch stages ----------------
        h1ps = {}

        def h1_mm(p, es):
            """h1 psum tiles for branch p from shifted elu es."""
            boff = 1280 * BR[p]
            tiles = []
            for mc in range(2):
                ps = psum_main.tile([128, 1024], f32, tag="pm")
                for kc in range(2):
                    lhsT = w_all[:, kc, boff + mc * 128:boff + (mc + 1) * 128]
                    for nk in range(2):
                        nc.tensor.matmul(
                            ps[:, nk * 512:(nk + 1) * 512],
                            lhsT=lhsT,
                            rhs=es[:, kc, nk * 512:(nk + 1) * 512],
                            start=(kc == 0), stop=(kc == 1))
                tiles.append(ps)
            h1ps[p] = tiles

        e1s = {}

        def e1_stage(p):
            """e1s = elu(h1)+1; consumes h1 psums."""
            e1 = e1_pool.tile([128, 2, S], bf16, tag="e1")
            for mc in range(2):
                ps = h1ps[p][mc]
                e = work.tile([128, S], bf16, tag="wk")
                r = work.tile([128, S], bf16, tag="wk")
                nc.scalar.activation(e, ps, AF.Exp, bias=bcol(p, "b1", mc))
                if cfg["h1relu"] == "scalar":
                    nc.scalar.activation(r, ps, AF.Relu, bias=bcol(p, "b1", mc))
                else:
                    nc.vector.tensor_scalar(r, ps, bcol(p, "b1", mc), 0.0,
                                            Op.add, Op.max)
                nc.vector.scalar_tensor_tensor(e1[:, mc, :], e, 1.0, r,
                                               Op.min, Op.add)
            e1s[p] = e1

        h2ps = {}

        def h2_mm(p):
            boff = 1280 * BR[p] + 256
            e1 = e1s[p]
            tiles = []
            for cc in range(2):
                ps_a = psum_main.tile([128, 1024], f32, tag="pm")
                ps_g = psum_main.tile([128, 1024], f32, tag="pm")
                for kc in range(2):
                    for half, ps in ((0, ps_a), (2, ps_g)):
                        lhsT = w_all[:, kc, boff + (half + cc) * 128:
                                     boff + (half + cc + 1) * 128]
                        for nk in range(2):
                            nc.tensor.matmul(
                                ps[:, nk * 512:(nk + 1) * 512],
                                lhsT=lhsT,
                                rhs=e1[:, kc, nk * 512:(nk + 1) * 512],
                                start=(kc == 0), stop=(kc == 1))
                tiles.append((ps_a, ps_g))
            h2ps[p] = tiles

        grs = {}

        def gr_stage(p, x3):
            """gr = x + (a+b2a)*0.5*(1+tanh((g+b2g)*0.5)); consumes h2 psums."""
            gr = gr_pool.tile([128, 2, S], bf16, tag="gr")
            for cc in range(2):
                ps_a, ps_g = h2ps[p][cc]
                tg = work.tile([128, S], bf16, tag="wk")
                ha = work.tile([128, S], bf16, tag="wk")
                u = work.tile([128, S], bf16, tag="wk")
                nc.scalar.activation(tg, ps_g, AF.Tanh,
                                     bias=bcol(p, "b2gh", cc), scale=0.5)
                nc.vector.tensor_scalar(ha, ps_a, bcol(p, "b2a", cc), 0.5,
                                        Op.add, Op.mult)
                nc.vector.scalar_tensor_tensor(u, tg, 1.0, ha, Op.add, Op.mult)
                eng(cfg["gr_engine"]).tensor_tensor(gr[:, cc, :], u, x3[:, cc, :],
                                                    Op.add)
            grs[p] = gr

        def nin_v():
            boff = 1280 * BR["v"] + 768
            gr = grs["v"]
            v_sb = work.tile([128, 4, S], bf16, tag="vsb", bufs=1)
            for mc in range(4):
                ps = psum_main.tile([128, 1024], f32, tag="pm")
                for kc in range(2):
                    lhsT = w_all[:, kc, boff + mc * 128:boff + (mc + 1) * 128]
                    for nk in range(2):
                        nc.tensor.matmul(
                            ps[:, nk * 512:(nk + 1) * 512],
                            lhsT=lhsT,
                            rhs=gr[:, kc, nk * 512:(nk + 1) * 512],
                            start=(kc == 0), stop=(kc == 1))
                nc.scalar.activation(v_sb[:, mc, :], ps, AF.Identity,
                                     bias=bnv[:, mc:mc + 1])
                nc.gpsimd.dma_start(out=vproj_dram[mc * 128:(mc + 1) * 128, :],
                                    in_=v_sb[:, mc, :])
                # v_aug[j][p2, n, u] = V_att[128j+p2, 64n+u];
                # V_att[s, d] = vproj[s//2, (s%2)*512 + d]
                for j in (2 * mc, 2 * mc + 1):
                    src = vproj_dram[64 * j:64 * j + 64, :]
                    src = src.rearrange("c (h n u) -> c h n u", h=2, n=NH)
                    nc.sync.dma_start(out=v_aug[:, j, :, 0:VS], in_=src)

        def nin_t(p):
            """Transposed nin for q/k -> qT_m/kT_m. Pairs (tp, tp+4) share a
            psum tile; one TT moves both with bias."""
            boff = 1280 * BR[p] + 768
            gr = grs[p]
            tgt = qT_m if p == "q" else kT_m
            bslot = 1 if p == "q" else 0
            for tp in range(4):
                ps = psum_main.tile([128, 1024], f32, tag="pm")
                for jj in range(2):
                    hw_p = tp + 4 * jj
                    for kc in range(2):
                        nc.tensor.matmul(
                            ps[:, jj * D:(jj + 1) * D],
                            lhsT=gr[:, kc, hw_p * 128:(hw_p + 1) * 128],
                            rhs=w_all[:, kc, boff:boff + D],
                            start=(kc == 0), stop=(kc == 1))
                # out[p, 2a+jj] = ps[p, jj*512+a] + bn[a]
                out_ap = tgt[:, tp, :].rearrange("p (a jj) -> p a jj", jj=2)
                in_ap = ps.rearrange("p (jj a) -> p a jj", jj=2)
                bn_ap = bnb[:, bslot, :].rearrange("p (a jj) -> p a jj", jj=2)
                eng(cfg["nin_bias_engine"]).tensor_tensor(
                    out_ap, in_ap, bn_ap, Op.add)

        # pipeline: emission order interleaves engines; deps do the rest
        h1_mm("v", eluk)
        h1_mm("k", eluk)
        e1_stage("v")
        h1_mm("q", eluq)
        e1_stage("k")
        h2_mm("v")
        e1_stage("q")
        gr_stage("v", xk)
        h2_mm("k")
        gr_stage("k", xk)
        nin_v()
        h2_mm("q")
        gr_stage("q", xq)
        nin_t("k")
        nin_t("q")

        # ---------------- attention ----------------
        ctx_br.close()  # release branch psum banks
        with ExitStack() as ctx_a:
            psum_att = ctx_a.enter_context(tc.tile_pool(name="psum_att", bufs=3, space="PSUM"))
            psum_pv = ctx_a.enter_context(tc.tile_pool(name="psum_pv", bufs=2, space="PSUM"))
            eT_pool = ctx_a.enter_context(tc.tile_pool(name="eT", bufs=3))
            att_small = ctx_a.enter_context(tc.tile_pool(name="att_small", bufs=3))

            GROUPS = [(0,), (1, 7), (2, 6), (3, 5), (4,)]
            G = {}
            off = 0
            for grp in GROUPS:
                for j in grp:
                    G[j] = off
                    off += S - 128 * j
            uls = {}

            for n in range(NH):
                tp, po = n // 2, 64 * (n % 2)
                eT = eT_pool.tile([128, 4608], bf16, tag="eT")
                for grp in GROUPS:
                    glen = sum(S - 128 * j for j in grp)
                    gbase = G[grp[0]]
                    ps = psum_att.tile([128, 1024], f32, tag="pm")
                    for j in grp:
                        off = G[j] - gbase
                        lhsT = kT_m[po:po + 64, tp, 128 * j:128 * (j + 1)]
                        for s1a, s1b in _split_psum_ranges(off, off + (S - 128 * j)):
                            nc.tensor.matmul(
                                ps[:, s1a:s1b],
                                lhsT=lhsT,
                                rhs=qT_m[po:po + 64, tp,
                                         128 * j + (s1a - off):128 * j + (s1b - off)],
                                start=True, stop=True)
                    nc.scalar.activation(eT[:, gbase:gbase + glen],
                                         ps[:, 0:glen], AF.Exp, scale=SCALE)
                    for j in grp:
                        eng(cfg["mask_engine"]).tensor_tensor(
                            eT[:, G[j]:G[j] + 128], eT[:, G[j]:G[j] + 128],
                            mask01, Op.mult)

                # PV: j-outer so each v_aug lhsT loads once
                pv0 = psum_pv.tile([VS + 1, 512], f32, tag="pv")
                pv1 = psum_pv.tile([VS + 1, 512], f32, tag="pv")
                for j in range(8):
                    lhsT = v_aug[:, j, n, :]
                    if j <= 3:
                        s1a = max(0, 128 * j)
                        nc.tensor.matmul(
                            pv0[:, s1a:512],
                            lhsT=lhsT,
                            rhs=eT[:, G[j] + (s1a - 128 * j):G[j] + (512 - 128 * j)],
                            start=(j == 0), stop=(j == 3))
                    s1a = max(512, 128 * j)
                    nc.tensor.matmul(
                        pv1[:, s1a - 512:512],
                        lhsT=lhsT,
                        rhs=eT[:, G[j] + (s1a - 128 * j):G[j] + (1024 - 128 * j)],
                        start=(j == 0), stop=(j == 7))
                nc.vector.memset(pv0[VS:VS + 1, 0:1], 1.0)

                ul = att_small.tile([VS + 1, 1024], f32, tag="ul", bufs=4)
                for c, pv in ((0, pv0), (1, pv1)):
                    eng(cfg["ul_engine"]).tensor_copy(
                        ul[:, c * 512:(c + 1) * 512], pv)
                g, b = n // 2, n % 2
                lb = 32 * g + 16 * b
                nc.sync.dma_start(out=lbuf[lb:lb + 16, :],
                                  in_=ul[VS:VS + 1, :])
                uls[n] = ul

                if n % 2 == 0:
                    continue
                # head pair (n-1, n) done: 1/l on partitions 32g..32g+31
                nc.vector.reciprocal(rbuf[32 * g:32 * g + 32, :],
                                     lbuf[32 * g:32 * g + 32, :])
                nc.gpsimd.dma_start(out=rbuf_dram[32 * g:32 * g + 32, :],
                                    in_=rbuf[32 * g:32 * g + 32, :])
                rflat = rbuf_dram.rearrange("a b -> (a b)")
                for nn in (n - 1, n):
                    bb_ = nn % 2
                    fin = att_small.tile([VS, 1024], f32, tag="fin", bufs=3)
                    rbc = att_small.tile([VS, 1024], f32, tag="rbc", bufs=4)
                    rd = rflat[(32 * g + 16 * bb_) * 64:
                               (32 * g + 16 * bb_) * 64 + 1024]
                    rsrc = bass.AP(tensor=rd.tensor, offset=rd.offset,
                                   ap=[[0, VS]] + list(rd.ap))
                    nc.gpsimd.dma_start(out=rbc, in_=rsrc)
                    eng(cfg["fin_engine"]).tensor_tensor(
                        fin, uls.pop(nn)[0:VS, :], rbc, Op.mult)
                    nc.sync.dma_start(out=out_d[VS * nn:VS * (nn + 1), :],
                                      in_=fin)

    nc.compile()
    return nc


_CACHE = {}


def _get_program(cfg_key=None):
    key = cfg_key or "default"
    if key not in _CACHE:
        _CACHE[key] = build_program(CFG)
    return _CACHE[key]


def make_in_map(inp, b):
    """Per-core input dict for batch b (weights host-packed/cast/bias-folded)."""
    import ml_dtypes
    wt = np.dtype(ml_dtypes.bfloat16)

    m = {
        "query": np.ascontiguousarray(inp["query"][b].reshape(C, S)).astype(wt),
        "key": np.ascontiguousarray(inp["key"][b].reshape(C, S)).astype(wt),
    }
    wall = np.zeros((C, 3 * 1280), dtype=wt)
    ball = np.zeros((128, 18), dtype=np.float32)
    BR = {"v": 0, "k": 1, "q": 2}
    for p in ("v", "k", "q"):
        src = "v" if p == "v" else p
        w1 = inp[f"{src}_gr_w1"].astype(wt)   # (C, C) row=cout
        w2 = inp[f"{src}_gr_w2"].astype(wt)   # (2C, C)
        wn = inp[f"{src}_nin_w"].astype(wt)   # (D, C)
        boff = 1280 * BR[p]
        wall[:, boff:boff + 256] = w1.T
        wall[:, boff + 256:boff + 768] = w2.T
        wall[:, boff + 768:boff + 1280] = wn.T
        # bias folding for the +1-shifted elu inputs
        b1a = inp[f"{src}_gr_b1"] - w1.astype(np.float32).sum(axis=1)
        b2a = inp[f"{src}_gr_b2"] - w2.astype(np.float32).sum(axis=1)
        b0 = 6 * BR[p]
        ball[:, b0 + 0:b0 + 2] = b1a.reshape(2, 128).T
        ball[:, b0 + 2:b0 + 4] = b2a[0:C].reshape(2, 128).T
        ball[:, b0 + 4:b0 + 6] = 0.5 * b2a[C:2 * C].reshape(2, 128).T
    m["wall"] = wall
    m["ball"] = ball
    m["bnv"] = np.ascontiguousarray(
        inp["v_nin_b"].reshape(4, 128).T).astype(np.float32)
    bnqk = np.zeros((2, 2 * D), dtype=np.float32)
    for slot, p in ((0, "k"), (1, "q")):
        bn = inp[f"{p}_nin_b"].astype(np.float32)
        bnqk[slot, 0::2] = bn
        bnqk[slot, 1::2] = bn
    m["bnqk"] = bnqk.reshape(-1)
    return m


def kernel(**inputs):
    from concourse.bass_utils import run_bass_kernel_spmd

    nc = _get_program()
    inp = {k: np.asarray(v, dtype=np.float32) for k, v in inputs.items()}

    in_maps = [make_in_map(inp, b) for b in range(N_CORES)]

    trace = bool(int(os.environ.get("BASS_KERNEL_TRACE", "0")))
    res = run_bass_kernel_spmd(nc, in_maps, core_ids=list(range(N_CORES)),
                               trace=trace)
    LAST_RUN["exec_time_ns"] = getattr(res, "exec_time_ns", None)
    LAST_RUN["results"] = res
    out = np.stack([res.results[i]["out"].reshape(D, 32, 32)
                    for i in range(N_CORES)])
    return out.astype(np.float32)


LAST_RUN = {}


if __name__ == "__main__":
    nc = build_program()
    print("compiled OK")


# revision 17
# speedup vs baseline: 1.1160x; 1.1160x over previous
"""Trainium2 Bass kernel for nn_CausalAttention (gated-resnet q/k/v projections
+ causal attention). Data-parallel over batch: 8 batches -> 8 NeuronCores.

Per-core computation (batch b):
  x_q = query[b] (C=256, S=1024)   x_k = key[b] (256, 1024)
  branch(p, x): e  = elu(x)
                h1 = W1 @ e + b1 ; e1 = elu(h1)
                h2 = W2 @ e1 + b2 ; a, g = split(h2)
                gr = x + a * sigmoid(g)
                o  = Wn @ gr + bn          (512, 1024) channel-major
  q = branch(q, x_q); k = branch(k, x_k); v = branch(k-input, x_k)
  att view: X_att[s, d] = X_cm[s//2, (s%2)*512 + d]  (flat reinterpretation)
  per head n (d = 64n..64n+63):
    scoresT[s2, s1] = sum_d K_att[s2,d] Q_att[s1,d]   (s2 causal blocks)
    eT = exp(scoresT/sqrt(512)) with strict-lower mask (s2 < s1)
    outT[vs, s1] = sum_s2 V_att[s2, 64n+vs] * eT[s2, s1] ; l[s1] = sum_s2 eT
    final[64n+vs, s1] = outT[vs, s1] / l[s1]   (l[0] patched to 1)

Implementation notes:
  - elu computed in shifted form elus = elu(x)+1 = relu(x) + min(exp(x),1);
    the -1 is folded into the next layer's bias host-side
    (b_adj = b - W.sum(axis=1)).
  - engines: exp/tanh/identity on ACT (one table), relu/ha on Pool (gpsimd),
    combines on DVE (bf16 2x), masks on DVE, ul copies on Pool.
  - branch stages emitted interleaved (v,k,q) so PE never starves.
"""

import os
import sys
import numpy as np

sys.path.insert(0, "/opt/trn_rl_repo")

C = 256
S = 1024
D = 512
NH = 8
KS = 64
VS = 64
SCALE = 1.0 / float(np.sqrt(512.0))
N_CORES = 8

# gpsimd elementwise measured ~14 cyc/elem + SBUF port contention with DVE:
# keep Pool for memset/affine_select/DMA issue only.
CFG = {
    "mask_engine": "vector",     # eT diag mask TT mult
    "ul_engine": "vector",       # psum->sbuf unnorm copy (psum: DVE/ACT only)
    "nin_bias_engine": "vector", # qT/kT bias add (psum: DVE/ACT only)
    "gr_engine": "vector",       # gr = u + x
    "fin_engine": "vector",      # fin = ul * rbc
    "h1relu": "scalar",          # relu(h1+b1) via ACT Relu | vector TS
}


def _split_psum_ranges(a, b, max_n=512):
    """Split [a, b) psum column range into chunks that don't cross 512-col
    bank boundaries and are <= max_n wide."""
    out = []
    while a < b:
        nxt = min(b, ((a // 512) + 1) * 512, a + max_n)
        out.append((a, nxt))
        a = nxt
    return out


def build_program(cfg=CFG):
    from contextlib import ExitStack

    import concourse.bacc as bacc
    import concourse.bass as bass
    import concourse.tile as tile
    from concourse import mybir
    from concourse.alu_op_type import AluOpType as Op

    f32 = mybir.dt.float32
    bf16 = mybir.dt.bfloat16
    AF = mybir.ActivationFunctionType

    nc = bacc.Bacc("TRN2", target_bir_lowering=False, debug=False,
                   num_devices=N_CORES)

    # ---------------- DRAM parameters ----------------
    query = nc.dram_tensor("query", [C, S], bf16, kind="ExternalInput").ap()
    key = nc.dram_tensor("key", [C, S], bf16, kind="ExternalInput").ap()
    # all weights packed [C, 3*1280]: per branch (v,k,q): w1T(256)|w2T(512)|wnT(512)
    wall = nc.dram_tensor("wall", [C, 3 * 1280], bf16, kind="ExternalInput").ap()
    # per-partition biases [128, 18]: per branch: b1_adj(2)|b2a_adj(2)|b2gh_adj(2)
    ball = nc.dram_tensor("ball", [128, 18], f32, kind="ExternalInput").ap()
    # v nin bias per-partition [128, 4]
    bnv_d = nc.dram_tensor("bnv", [128, 4], f32, kind="ExternalInput").ap()
    # q|k nin biases, interleaved [slot, 2a+jj] = bn[slot][a], for broadcast
    bnqk = nc.dram_tensor("bnqk", [2 * 2 * D], f32, kind="ExternalInput").ap()
    out_d = nc.dram_tensor("out", [D, S], f32, kind="ExternalOutput").ap()

    BR = {"v": 0, "k": 1, "q": 2}  # branch order / wall offsets

    def eng(name):
        return getattr(nc, name)

    with tile.TileContext(nc) as tc, ExitStack() as ctx:
        persist = ctx.enter_context(tc.tile_pool(name="persist", bufs=1))
        ctx_br = ExitStack()
        psum_main = ctx_br.enter_context(tc.tile_pool(name="psum_br", bufs=4, space="PSUM"))
        dram_pool = ctx.enter_context(tc.tile_pool(name="dram", bufs=1, space="DRAM"))

        # persistent tiles
        xq = persist.tile([128, 2, S], bf16)
        xk = persist.tile([128, 2, S], bf16)
        eluq = persist.tile([128, 2, S], bf16)   # elu(x)+1
        eluk = persist.tile([128, 2, S], bf16)
        w_all = persist.tile([128, 2, 3 * 1280], bf16)   # [cin%128, cin//128, col]
        bb = persist.tile([128, 18], f32)
        bnv = persist.tile([128, 4], f32)
        bnb = persist.tile([128, 2, 2 * D], f32)  # broadcast q|k nin bias (interleaved)
        qT_m = persist.tile([128, 4, S], bf16)   # Q^T_att: [dd%128, dd//128, s]
        kT_m = persist.tile([128, 4, S], bf16)
        v_aug = persist.tile([128, 8, NH, VS + 1], bf16)  # [s%128, s//128, n, vs|1]
        mask01 = persist.tile([128, 128], bf16)  # [t2, t1] = 1.0 if t1 > t2 else 0
        # l and 1/l packed [16 rows x 64] per head; pair g owns partitions
        # 32g..32g+31 so the reciprocal's start partition is 32-aligned and its
        # free size is only 64 (reciprocal cost scales with free size)
        lbuf = persist.tile([128, 64], f32)
        rbuf = persist.tile([128, 64], f32)

        vproj_dram = dram_pool.tile([D, S], bf16)
        rbuf_dram = dram_pool.tile([128, 64], f32)

        # PE warm-up (p-state ramp): back-to-back matmuls on scratch data
        warm = persist.tile([128, 512], bf16, name="warm")
        nc.vector.memset(warm, 0.5)
        wps = psum_main.tile([128, 1024], f32, tag="pm", name="wps")
        for _ in range(12):
            nc.tensor.matmul(wps[:, 0:512], lhsT=warm[:, 0:128],
                             rhs=warm, start=True, stop=True)

        # ---------------- input + weight DMAs ----------------
        for cc in range(2):
            nc.sync.dma_start(out=xk[:, cc, :], in_=key[cc * 128:(cc + 1) * 128, :])
            nc.sync.dma_start(out=xq[:, cc, :], in_=query[cc * 128:(cc + 1) * 128, :])
        wall4 = wall.rearrange("(kc p) c -> p kc c", p=128)
        for b in range(3):
            nc.sync.dma_start(out=w_all[:, :, b * 1280:(b + 1) * 1280],
                              in_=wall4[:, :, b * 1280:(b + 1) * 1280])
        nc.sync.dma_start(out=bb, in_=ball)
        nc.sync.dma_start(out=bnv, in_=bnv_d)
        bn_bcast = bass.AP(tensor=bnqk.tensor, offset=bnqk.offset,
                           ap=[[0, 128]] + list(bnqk.ap))
        nc.sync.dma_start(out=bnb.rearrange("p a b -> p (a b)"), in_=bn_bcast)

        # mask + v_aug ones columns
        nc.gpsimd.memset(mask01, 1.0)
        nc.gpsimd.affine_select(
            out=mask01, in_=mask01, compare_op=Op.is_ge, fill=0.0,
            base=-1, pattern=[[1, 128]], channel_multiplier=-1)
        nc.gpsimd.memset(v_aug[:, :, :, VS:VS + 1], 1.0)

        # bias column helpers: bb cols per branch b0=6*BR[p]:
        #   b1_adj: b0+0,b0+1 ; b2a_adj: b0+2,b0+3 ; b2gh_adj: b0+4,b0+5
        def bcol(p, kind, i):
            b0 = 6 * BR[p]
            off = {"b1": 0, "b2a": 2, "b2gh": 4}[kind]
            return bb[:, b0 + off + i:b0 + off + i + 1]

        work = ctx.enter_context(tc.tile_pool(name="work", bufs=8))
        e1_pool = ctx.enter_context(tc.tile_pool(name="e1", bufs=2))
        gr_pool = ctx.enter_context(tc.tile_pool(name="gr", bufs=3))

        # ---------------- input elus (shifted): elus = relu(x)+min(exp(x),1)
        def elu_in(x3, dst3, cc):
            r = work.tile([128, S], bf16, tag="wk", bufs=8)
            e = work.tile([128, S], bf16, tag="wk", bufs=8)
            nc.scalar.activation(r, x3[:, cc, :], AF.Relu)
            nc.scalar.activation(e, x3[:, cc, :], AF.Exp)
            nc.vector.scalar_tensor_tensor(dst3[:, cc, :], e, 1.0, r,
                                           Op.min, Op.add)

        elu_in(xk, eluk, 0)
        elu_in(xk, eluk, 1)
        elu_in(xq, eluq, 0)
        elu_in(xq, eluq, 1)

        # ---------------- branch stages ----------------
        h1ps = {}

        def h1_mm(p, es):
            """h1 psum tiles for branch p from shifted elu es."""
            boff = 1280 * BR[p]
            tiles = []
            for mc in range(2):
                ps = psum_main.tile([128, 1024], f32, tag="pm")
                for kc in range(2):
                    lhsT = w_all[:, kc, boff + mc * 128:boff + (mc + 1) * 128]
                    for nk in range(2):
                        nc.tensor.matmul(
                            ps[:, nk * 512:(nk + 1) * 512],
                            lhsT=lhsT,
                            rhs=es[:, kc, nk * 512:(nk + 1) * 512],
                            start=(kc == 0), stop=(kc == 1))
                tiles.append(ps)
            h1ps[p] = tiles

        e1s = {}

        def e1_stage(p):
            """e1s = elu(h1)+1; consumes h1 psums."""
            e1 = e1_pool.tile([128, 2, S], bf16, tag="e1")
            for mc in range(2):
                ps = h1ps[p][mc]
                e = work.tile([128, S], bf16, tag="wk")
                r = work.tile([128, S], bf16, tag="wk")
                nc.scalar.activation(e, ps, AF.Exp, bias=bcol(p, "b1", mc))
                if cfg["h1relu"] == "scalar":
                    nc.scalar.activation(r, ps, AF.Relu, bias=bcol(p, "b1", mc))
                else:
                    nc.vector.tensor_scalar(r, ps, bcol(p, "b1", mc), 0.0,
                                            Op.add, Op.max)
                nc.vector.scalar_tensor_tensor(e1[:, mc, :], e, 1.0, r,
                                               Op.min, Op.add)
            e1s[p] = e1

        h2ps = {}

        def h2_mm(p):
            boff = 1280 * BR[p] + 256
            e1 = e1s[p]
            tiles = []
            for cc in range(2):
                ps_a = psum_main.tile([128, 1024], f32, tag="pm")
                ps_g = psum_main.tile([128, 1024], f32, tag="pm")
                for kc in range(2):
                    for half, ps in ((0, ps_a), (2, ps_g)):
                        lhsT = w_all[:, kc, boff + (half + cc) * 128:
                                     boff + (half + cc + 1) * 128]
                        for nk in range(2):
                            nc.tensor.matmul(
                                ps[:, nk * 512:(nk + 1) * 512],
                                lhsT=lhsT,
                                rhs=e1[:, kc, nk * 512:(nk + 1) * 512],
                                start=(kc == 0), stop=(kc == 1))
                tiles.append((ps_a, ps_g))
            h2ps[p] = tiles

        grs = {}

        def gr_stage(p, x3):
            """gr = x + (a+b2a)*0.5*(1+tanh((g+b2g)*0.5)); consumes h2 psums."""
            gr = gr_pool.tile([128, 2, S], bf16, tag="gr")
            for cc in range(2):
                ps_a, ps_g = h2ps[p][cc]
                tg = work.tile([128, S], bf16, tag="wk")
                ha = work.tile([128, S], bf16, tag="wk")
                u = work.tile([128, S], bf16, tag="wk")
                nc.scalar.activation(tg, ps_g, AF.Tanh,
                                     bias=bcol(p, "b2gh", cc), scale=0.5)
                nc.vector.tensor_scalar(ha, ps_a, bcol(p, "b2a", cc), 0.5,
                                        Op.add, Op.mult)
                nc.vector.scalar_tensor_tensor(u, tg, 1.0, ha, Op.add, Op.mult)
                eng(cfg["gr_engine"]).tensor_tensor(gr[:, cc, :], u, x3[:, cc, :],
                                                    Op.add)
            grs[p] = gr

        def nin_v():
            boff = 1280 * BR["v"] + 768
            gr = grs["v"]
            v_sb = work.tile([128, 4, S], bf16, tag="vsb", bufs=1)
            for mc in range(4):
                ps = psum_main.tile([128, 1024], f32, tag="pm")
                for kc in range(2):
                    lhsT = w_all[:, kc, boff + mc * 128:boff + (mc + 1) * 128]
                    for nk in range(2):
                        nc.tensor.matmul(
                            ps[:, nk * 512:(nk + 1) * 512],
                            lhsT=lhsT,
                            rhs=gr[:, kc, nk * 512:(nk + 1) * 512],
                            start=(kc == 0), stop=(kc == 1))
                nc.scalar.activation(v_sb[:, mc, :], ps, AF.Identity,
                                     bias=bnv[:, mc:mc + 1])
                nc.gpsimd.dma_start(out=vproj_dram[mc * 128:(mc + 1) * 128, :],
                                    in_=v_sb[:, mc, :])
                # v_aug[j][p2, n, u] = V_att[128j+p2, 64n+u];
                # V_att[s, d] = vproj[s//2, (s%2)*512 + d]
                for j in (2 * mc, 2 * mc + 1):
                    src = vproj_dram[64 * j:64 * j + 64, :]
                    src = src.rearrange("c (h n u) -> c h n u", h=2, n=NH)
                    nc.sync.dma_start(out=v_aug[:, j, :, 0:VS], in_=src)

        def nin_t(p, tps=(0, 1, 2, 3), pool=None):
            """Transposed nin for q/k -> qT_m/kT_m. Pairs (tp, tp+4) share a
            psum tile; one TT moves both with bias."""
            boff = 1280 * BR[p] + 768
            gr = grs[p]
            tgt = qT_m if p == "q" else kT_m
            bslot = 1 if p == "q" else 0
            for tp in tps:
                ps = (pool or psum_main).tile([128, 1024], f32, tag="pm")
                for jj in range(2):
                    hw_p = tp + 4 * jj
                    for kc in range(2):
                        nc.tensor.matmul(
                            ps[:, jj * D:(jj + 1) * D],
                            lhsT=gr[:, kc, hw_p * 128:(hw_p + 1) * 128],
                            rhs=w_all[:, kc, boff:boff + D],
                            start=(kc == 0), stop=(kc == 1))
                # out[p, 2a+jj] = ps[p, jj*512+a] + bn[a]
                out_ap = tgt[:, tp, :].rearrange("p (a jj) -> p a jj", jj=2)
                in_ap = ps.rearrange("p (jj a) -> p a jj", jj=2)
                bn_ap = bnb[:, bslot, :].rearrange("p (a jj) -> p a jj", jj=2)
                eng(cfg["nin_bias_engine"]).tensor_tensor(
                    out_ap, in_ap, bn_ap, Op.add)

        # pipeline: emission order interleaves engines; deps do the rest
        h1_mm("v", eluk)
        h1_mm("k", eluk)
        e1_stage("v")
        h1_mm("q", eluq)
        e1_stage("k")
        h2_mm("v")
        e1_stage("q")
        gr_stage("v", xk)
        h2_mm("k")
        gr_stage("k", xk)
        nin_v()
        h2_mm("q")
        gr_stage("q", xq)
        nin_t("k")

        # ---------------- attention (software-pipelined with nin_q) --------
        ctx_br.close()  # release branch psum banks; nin_q uses psum_att
        with ExitStack() as ctx_a:
            psum_att = ctx_a.enter_context(tc.tile_pool(name="psum_att", bufs=3, space="PSUM"))
            psum_pv = ctx_a.enter_context(tc.tile_pool(name="psum_pv", bufs=2, space="PSUM"))
            eT_pool = ctx_a.enter_context(tc.tile_pool(name="eT", bufs=3))
            att_small = ctx_a.enter_context(tc.tile_pool(name="att_small", bufs=3))

            GROUPS = [(0,), (1, 7), (2, 6), (3, 5), (4,)]
            G = {}
            off = 0
            for grp in GROUPS:
                for j in grp:
                    G[j] = off
                    off += S - 128 * j
            uls = {}

            def scores_head(n):
                tp, po = n // 2, 64 * (n % 2)
                eT = eT_pool.tile([128, 4608], bf16, tag="eT")
                for grp in GROUPS:
                    glen = sum(S - 128 * j for j in grp)
                    gbase = G[grp[0]]
                    ps = psum_att.tile([128, 1024], f32, tag="pm")
                    for j in grp:
                        off = G[j] - gbase
                        lhsT = kT_m[po:po + 64, tp, 128 * j:128 * (j + 1)]
                        for s1a, s1b in _split_psum_ranges(off, off + (S - 128 * j)):
                            nc.tensor.matmul(
                                ps[:, s1a:s1b],
                                lhsT=lhsT,
                                rhs=qT_m[po:po + 64, tp,
                                         128 * j + (s1a - off):128 * j + (s1b - off)],
                                start=True, stop=True)
                    nc.scalar.activation(eT[:, gbase:gbase + glen],
                                         ps[:, 0:glen], AF.Exp, scale=SCALE)
                    for j in grp:
                        eng(cfg["mask_engine"]).tensor_tensor(
                            eT[:, G[j]:G[j] + 128], eT[:, G[j]:G[j] + 128],
                            mask01, Op.mult)
                return eT

            def pv_head(n, eT):
                # PV: j-outer so each v_aug lhsT loads once
                pv0 = psum_pv.tile([VS + 1, 512], f32, tag="pv")
                pv1 = psum_pv.tile([VS + 1, 512], f32, tag="pv")
                for j in range(8):
                    lhsT = v_aug[:, j, n, :]
                    if j <= 3:
                        s1a = max(0, 128 * j)
                        nc.tensor.matmul(
                            pv0[:, s1a:512],
                            lhsT=lhsT,
                            rhs=eT[:, G[j] + (s1a - 128 * j):G[j] + (512 - 128 * j)],
                            start=(j == 0), stop=(j == 3))
                    s1a = max(512, 128 * j)
                    nc.tensor.matmul(
                        pv1[:, s1a - 512:512],
                        lhsT=lhsT,
                        rhs=eT[:, G[j] + (s1a - 128 * j):G[j] + (1024 - 128 * j)],
                        start=(j == 0), stop=(j == 7))
                nc.vector.memset(pv0[VS:VS + 1, 0:1], 1.0)

                ul = att_small.tile([VS + 1, 1024], f32, tag="ul", bufs=4)
                for c, pv in ((0, pv0), (1, pv1)):
                    eng(cfg["ul_engine"]).tensor_copy(
                        ul[:, c * 512:(c + 1) * 512], pv)
                g, b = n // 2, n % 2
                lb = 32 * g + 16 * b
                nc.sync.dma_start(out=lbuf[lb:lb + 16, :],
                                  in_=ul[VS:VS + 1, :])
                uls[n] = ul

                if n % 2 == 0:
                    return
                # head pair (n-1, n) done: 1/l on partitions 32g..32g+31
                nc.vector.reciprocal(rbuf[32 * g:32 * g + 32, :],
                                     lbuf[32 * g:32 * g + 32, :])
                nc.gpsimd.dma_start(out=rbuf_dram[32 * g:32 * g + 32, :],
                                    in_=rbuf[32 * g:32 * g + 32, :])
                rflat = rbuf_dram.rearrange("a b -> (a b)")
                for nn in (n - 1, n):
                    bb_ = nn % 2
                    fin = att_small.tile([VS, 1024], f32, tag="fin", bufs=3)
                    rbc = att_small.tile([VS, 1024], f32, tag="rbc", bufs=4)
                    rd = rflat[(32 * g + 16 * bb_) * 64:
                               (32 * g + 16 * bb_) * 64 + 1024]
                    rsrc = bass.AP(tensor=rd.tensor, offset=rd.offset,
                                   ap=[[0, VS]] + list(rd.ap))
                    nc.gpsimd.dma_start(out=rbc, in_=rsrc)
                    eng(cfg["fin_engine"]).tensor_tensor(
                        fin, uls.pop(nn)[0:VS, :], rbc, Op.mult)
                    nc.sync.dma_start(out=out_d[VS * nn:VS * (nn + 1), :],
                                      in_=fin)

            # emission: ninq(tp) feeds heads 2tp,2tp+1; scores run one head
            # ahead of PV so the in-order PE queue never blocks on eT
            eTs = {}
            nin_t("q", tps=(0,), pool=psum_att)
            eTs[0] = scores_head(0)
            nin_t("q", tps=(1,), pool=psum_att)
            eTs[1] = scores_head(1)
            nin_t("q", tps=(2,), pool=psum_att)
            for n in range(NH):
                if n + 2 < NH:
                    eTs[n + 2] = scores_head(n + 2)
                if n == 0:
                    nin_t("q", tps=(3,), pool=psum_att)
                pv_head(n, eTs.pop(n))

    nc.compile()
    return nc


_CACHE = {}


def _get_program(cfg_key=None):
    key = cfg_key or "default"
    if key not in _CACHE:
        _CACHE[key] = build_program(CFG)
    return _CACHE[key]


def make_in_map(inp, b):
    """Per-core input dict for batch b (weights host-packed/cast/bias-folded)."""
    import ml_dtypes
    wt = np.dtype(ml_dtypes.bfloat16)

    m = {
        "query": np.ascontiguousarray(inp["query"][b].reshape(C, S)).astype(wt),
        "key": np.ascontiguousarray(inp["key"][b].reshape(C, S)).astype(wt),
    }
    wall = np.zeros((C, 3 * 1280), dtype=wt)
    ball = np.zeros((128, 18), dtype=np.float32)
    BR = {"v": 0, "k": 1, "q": 2}
    for p in ("v", "k", "q"):
        src = "v" if p == "v" else p
        w1 = inp[f"{src}_gr_w1"].astype(wt)   # (C, C) row=cout
        w2 = inp[f"{src}_gr_w2"].astype(wt)   # (2C, C)
        wn = inp[f"{src}_nin_w"].astype(wt)   # (D, C)
        boff = 1280 * BR[p]
        wall[:, boff:boff + 256] = w1.T
        wall[:, boff + 256:boff + 768] = w2.T
        wall[:, boff + 768:boff + 1280] = wn.T
        # bias folding for the +1-shifted elu inputs
        b1a = inp[f"{src}_gr_b1"] - w1.astype(np.float32).sum(axis=1)
        b2a = inp[f"{src}_gr_b2"] - w2.astype(np.float32).sum(axis=1)
        b0 = 6 * BR[p]
        ball[:, b0 + 0:b0 + 2] = b1a.reshape(2, 128).T
        ball[:, b0 + 2:b0 + 4] = b2a[0:C].reshape(2, 128).T
        ball[:, b0 + 4:b0 + 6] = 0.5 * b2a[C:2 * C].reshape(2, 128).T
    m["wall"] = wall
    m["ball"] = ball
    m["bnv"] = np.ascontiguousarray(
        inp["v_nin_b"].reshape(4, 128).T).astype(np.float32)
    bnqk = np.zeros((2, 2 * D), dtype=np.float32)
    for slot, p in ((0, "k"), (1, "q")):
        bn = inp[f"{p}_nin_b"].astype(np.float32)
        bnqk[slot, 0::2] = bn
        bnqk[slot, 1::2] = bn
    m["bnqk"] = bnqk.reshape(-1)
    return m


def kernel(**inputs):
    from concourse.bass_utils import run_bass_kernel_spmd

    nc = _get_program()
    inp = {k: np.asarray(v, dtype=np.float32) for k, v in inputs.items()}

    in_maps = [make_in_map(inp, b) for b in range(N_CORES)]

    trace = bool(int(os.environ.get("BASS_KERNEL_TRACE", "0")))
    res = run_bass_kernel_spmd(nc, in_maps, core_ids=list(range(N_CORES)),
                               trace=trace)
    LAST_RUN["exec_time_ns"] = getattr(res, "exec_time_ns", None)
    LAST_RUN["results"] = res
    out = np.stack([res.results[i]["out"].reshape(D, 32, 32)
                    for i in range(N_CORES)])
    return out.astype(np.float32)


LAST_RUN = {}


if __name__ == "__main__":
    nc = build_program()
    print("compiled OK")


# revision 18
# speedup vs baseline: 1.1230x; 1.0063x over previous
"""Trainium2 Bass kernel for nn_CausalAttention (gated-resnet q/k/v projections
+ causal attention). Data-parallel over batch: 8 batches -> 8 NeuronCores.

Per-core computation (batch b):
  x_q = query[b] (C=256, S=1024)   x_k = key[b] (256, 1024)
  branch(p, x): e  = elu(x)
                h1 = W1 @ e + b1 ; e1 = elu(h1)
                h2 = W2 @ e1 + b2 ; a, g = split(h2)
                gr = x + a * sigmoid(g)
                o  = Wn @ gr + bn          (512, 1024) channel-major
  q = branch(q, x_q); k = branch(k, x_k); v = branch(k-input, x_k)
  att view: X_att[s, d] = X_cm[s//2, (s%2)*512 + d]  (flat reinterpretation)
  per head n (d = 64n..64n+63):
    scoresT[s2, s1] = sum_d K_att[s2,d] Q_att[s1,d]   (s2 causal blocks)
    eT = exp(scoresT/sqrt(512)) with strict-lower mask (s2 < s1)
    outT[vs, s1] = sum_s2 V_att[s2, 64n+vs] * eT[s2, s1] ; l[s1] = sum_s2 eT
    final[64n+vs, s1] = outT[vs, s1] / l[s1]   (l[0] patched to 1)

Implementation notes:
  - elu computed in shifted form elus = elu(x)+1 = relu(x) + min(exp(x),1);
    the -1 is folded into the next layer's bias host-side
    (b_adj = b - W.sum(axis=1)).
  - engines: exp/tanh/identity on ACT (one table), relu/ha on Pool (gpsimd),
    combines on DVE (bf16 2x), masks on DVE, ul copies on Pool.
  - branch stages emitted interleaved (v,k,q) so PE never starves.
"""

import os
import sys
import numpy as np

sys.path.insert(0, "/opt/trn_rl_repo")

C = 256
S = 1024
D = 512
NH = 8
KS = 64
VS = 64
SCALE = 1.0 / float(np.sqrt(512.0))
N_CORES = 8

# gpsimd elementwise measured ~14 cyc/elem + SBUF port contention with DVE:
# keep Pool for memset/affine_select/DMA issue only.
CFG = {
    "mask_engine": "vector",     # eT diag mask TT mult
    "ul_engine": "vector",       # psum->sbuf unnorm copy (psum: DVE/ACT only)
    "nin_bias_engine": "vector", # qT/kT bias add (psum: DVE/ACT only)
    "gr_engine": "vector",       # gr = u + x
    "fin_engine": "vector",      # fin = ul * rbc
    "h1relu": "scalar",          # relu(h1+b1) via ACT Relu | vector TS
}


def _split_psum_ranges(a, b, max_n=512):
    """Split [a, b) psum column range into chunks that don't cross 512-col
    bank boundaries and are <= max_n wide."""
    out = []
    while a < b:
        nxt = min(b, ((a // 512) + 1) * 512, a + max_n)
        out.append((a, nxt))
        a = nxt
    return out


def build_program(cfg=CFG):
    from contextlib import ExitStack

    import concourse.bacc as bacc
    import concourse.bass as bass
    import concourse.tile as tile
    from concourse import mybir
    from concourse.alu_op_type import AluOpType as Op

    f32 = mybir.dt.float32
    bf16 = mybir.dt.bfloat16
    AF = mybir.ActivationFunctionType

    nc = bacc.Bacc("TRN2", target_bir_lowering=False, debug=False,
                   num_devices=N_CORES)

    # ---------------- DRAM parameters ----------------
    query = nc.dram_tensor("query", [C, S], bf16, kind="ExternalInput").ap()
    key = nc.dram_tensor("key", [C, S], bf16, kind="ExternalInput").ap()
    # all weights packed [C, 3*1280]: per branch (v,k,q): w1T(256)|w2T(512)|wnT(512)
    wall = nc.dram_tensor("wall", [C, 3 * 1280], bf16, kind="ExternalInput").ap()
    # per-partition biases [128, 18]: per branch: b1_adj(2)|b2a_adj(2)|b2gh_adj(2)
    ball = nc.dram_tensor("ball", [128, 18], f32, kind="ExternalInput").ap()
    # v nin bias per-partition [128, 4]
    bnv_d = nc.dram_tensor("bnv", [128, 4], f32, kind="ExternalInput").ap()
    # q|k nin biases, interleaved [slot, 2a+jj] = bn[slot][a], for broadcast
    bnqk = nc.dram_tensor("bnqk", [2 * 2 * D], f32, kind="ExternalInput").ap()
    out_d = nc.dram_tensor("out", [D, S], f32, kind="ExternalOutput").ap()

    BR = {"v": 0, "k": 1, "q": 2}  # branch order / wall offsets

    def eng(name):
        return getattr(nc, name)

    with tile.TileContext(nc) as tc, ExitStack() as ctx:
        persist = ctx.enter_context(tc.tile_pool(name="persist", bufs=1))
        ctx_br = ExitStack()
        psum_main = ctx_br.enter_context(tc.tile_pool(name="psum_br", bufs=4, space="PSUM"))
        dram_pool = ctx.enter_context(tc.tile_pool(name="dram", bufs=1, space="DRAM"))

        # persistent tiles
        xq = persist.tile([128, 2, S], bf16)
        xk = persist.tile([128, 2, S], bf16)
        eluq = persist.tile([128, 2, S], bf16)   # elu(x)+1
        eluk = persist.tile([128, 2, S], bf16)
        w_all = persist.tile([128, 2, 3 * 1280], bf16)   # [cin%128, cin//128, col]
        bb = persist.tile([128, 18], f32)
        bnv = persist.tile([128, 4], f32)
        bnb = persist.tile([128, 2, 2 * D], f32)  # broadcast q|k nin bias (interleaved)
        qT_m = persist.tile([128, 4, S], bf16)   # Q^T_att: [dd%128, dd//128, s]
        kT_m = persist.tile([128, 4, S], bf16)
        v_aug = persist.tile([128, 8, NH, VS + 1], bf16)  # [s%128, s//128, n, vs|1]
        mask01 = persist.tile([128, 128], bf16)  # [t2, t1] = 1.0 if t1 > t2 else 0
        # l and 1/l packed [16 rows x 64] per head; pair g owns partitions
        # 32g..32g+31 so the reciprocal's start partition is 32-aligned and its
        # free size is only 64 (reciprocal cost scales with free size)
        lbuf = persist.tile([128, 64], f32)
        rbuf = persist.tile([128, 64], f32)

        vproj_dram = dram_pool.tile([D, S], bf16)
        rbuf_dram = dram_pool.tile([128, 64], f32)

        # PE warm-up (p-state ramp): back-to-back matmuls on scratch data
        warm = persist.tile([128, 512], bf16, name="warm")
        nc.vector.memset(warm, 0.5)
        wps = psum_main.tile([128, 1024], f32, tag="pm", name="wps")
        for _ in range(12):
            nc.tensor.matmul(wps[:, 0:512], lhsT=warm[:, 0:128],
                             rhs=warm, start=True, stop=True)

        # ---------------- input + weight DMAs ----------------
        for cc in range(2):
            nc.sync.dma_start(out=xk[:, cc, :], in_=key[cc * 128:(cc + 1) * 128, :])
            nc.sync.dma_start(out=xq[:, cc, :], in_=query[cc * 128:(cc + 1) * 128, :])
        wall4 = wall.rearrange("(kc p) c -> p kc c", p=128)
        for b in range(3):
            nc.sync.dma_start(out=w_all[:, :, b * 1280:(b + 1) * 1280],
                              in_=wall4[:, :, b * 1280:(b + 1) * 1280])
        nc.sync.dma_start(out=bb, in_=ball)
        nc.sync.dma_start(out=bnv, in_=bnv_d)
        bn_bcast = bass.AP(tensor=bnqk.tensor, offset=bnqk.offset,
                           ap=[[0, 128]] + list(bnqk.ap))
        nc.sync.dma_start(out=bnb.rearrange("p a b -> p (a b)"), in_=bn_bcast)

        # mask + v_aug ones columns
        nc.gpsimd.memset(mask01, 1.0)
        nc.gpsimd.affine_select(
            out=mask01, in_=mask01, compare_op=Op.is_ge, fill=0.0,
            base=-1, pattern=[[1, 128]], channel_multiplier=-1)
        nc.gpsimd.memset(v_aug[:, :, :, VS:VS + 1], 1.0)

        # bias column helpers: bb cols per branch b0=6*BR[p]:
        #   b1_adj: b0+0,b0+1 ; b2a_adj: b0+2,b0+3 ; b2gh_adj: b0+4,b0+5
        def bcol(p, kind, i):
            b0 = 6 * BR[p]
            off = {"b1": 0, "b2ah": 2, "b2gh": 4}[kind]
            return bb[:, b0 + off + i:b0 + off + i + 1]

        work = ctx.enter_context(tc.tile_pool(name="work", bufs=8))
        e1_pool = ctx.enter_context(tc.tile_pool(name="e1", bufs=2))
        gr_pool = ctx.enter_context(tc.tile_pool(name="gr", bufs=3))

        # ---------------- input elus (shifted): elus = relu(x)+min(exp(x),1)
        def elu_in(x3, dst3, cc):
            # elus = elu(x)+1 = max(x+1, min(exp(x), 1))  (1+x <= e^x)
            e = work.tile([128, S], bf16, tag="wk", bufs=8)
            t = work.tile([128, S], bf16, tag="wk", bufs=8)
            nc.scalar.activation(e, x3[:, cc, :], AF.Exp)
            nc.vector.tensor_scalar_min(t, e, 1.0)
            nc.vector.scalar_tensor_tensor(dst3[:, cc, :], x3[:, cc, :], 1.0,
                                           t, Op.add, Op.max)

        elu_in(xk, eluk, 0)
        elu_in(xk, eluk, 1)
        elu_in(xq, eluq, 0)
        elu_in(xq, eluq, 1)

        # ---------------- branch stages ----------------
        h1ps = {}

        def h1_mm(p, es):
            """h1 psum tiles for branch p from shifted elu es."""
            boff = 1280 * BR[p]
            tiles = []
            for mc in range(2):
                ps = psum_main.tile([128, 1024], f32, tag="pm")
                for kc in range(2):
                    lhsT = w_all[:, kc, boff + mc * 128:boff + (mc + 1) * 128]
                    for nk in range(2):
                        nc.tensor.matmul(
                            ps[:, nk * 512:(nk + 1) * 512],
                            lhsT=lhsT,
                            rhs=es[:, kc, nk * 512:(nk + 1) * 512],
                            start=(kc == 0), stop=(kc == 1))
                tiles.append(ps)
            h1ps[p] = tiles

        e1s = {}

        def e1_stage(p):
            """e1s = elu(h1)+1; consumes h1 psums."""
            e1 = e1_pool.tile([128, 2, S], bf16, tag="e1")
            for mc in range(2):
                ps = h1ps[p][mc]
                e = work.tile([128, S], bf16, tag="wk")
                r = work.tile([128, S], bf16, tag="wk")
                nc.scalar.activation(e, ps, AF.Exp, bias=bcol(p, "b1", mc))
                if cfg["h1relu"] == "scalar":
                    nc.scalar.activation(r, ps, AF.Relu, bias=bcol(p, "b1", mc))
                else:
                    nc.vector.tensor_scalar(r, ps, bcol(p, "b1", mc), 0.0,
                                            Op.add, Op.max)
                nc.vector.scalar_tensor_tensor(e1[:, mc, :], e, 1.0, r,
                                               Op.min, Op.add)
            e1s[p] = e1

        h2ps = {}

        def h2_mm(p):
            boff = 1280 * BR[p] + 256
            e1 = e1s[p]
            tiles = []
            for cc in range(2):
                ps_a = psum_main.tile([128, 1024], f32, tag="pm")
                ps_g = psum_main.tile([128, 1024], f32, tag="pm")
                for kc in range(2):
                    for half, ps in ((0, ps_a), (2, ps_g)):
                        lhsT = w_all[:, kc, boff + (half + cc) * 128:
                                     boff + (half + cc + 1) * 128]
                        for nk in range(2):
                            nc.tensor.matmul(
                                ps[:, nk * 512:(nk + 1) * 512],
                                lhsT=lhsT,
                                rhs=e1[:, kc, nk * 512:(nk + 1) * 512],
                                start=(kc == 0), stop=(kc == 1))
                tiles.append((ps_a, ps_g))
            h2ps[p] = tiles

        grs = {}

        def gr_stage(p, x3):
            """gr = x + (a+b2a)*0.5*(1+tanh((g+b2g)*0.5)); consumes h2 psums."""
            gr = gr_pool.tile([128, 2, S], bf16, tag="gr")
            for cc in range(2):
                ps_a, ps_g = h2ps[p][cc]
                tg = work.tile([128, S], bf16, tag="wk")
                ha = work.tile([128, S], bf16, tag="wk")
                u = work.tile([128, S], bf16, tag="wk")
                nc.scalar.activation(tg, ps_g, AF.Tanh,
                                     bias=bcol(p, "b2gh", cc), scale=0.5)
                nc.scalar.activation(ha, ps_a, AF.Identity,
                                     bias=bcol(p, "b2ah", cc), scale=0.5)
                nc.vector.scalar_tensor_tensor(u, tg, 1.0, ha, Op.add, Op.mult)
                eng(cfg["gr_engine"]).tensor_tensor(gr[:, cc, :], u, x3[:, cc, :],
                                                    Op.add)
            grs[p] = gr

        def nin_v():
            boff = 1280 * BR["v"] + 768
            gr = grs["v"]
            v_sb = work.tile([128, 4, S], bf16, tag="vsb", bufs=1)
            for mc in range(4):
                ps = psum_main.tile([128, 1024], f32, tag="pm")
                for kc in range(2):
                    lhsT = w_all[:, kc, boff + mc * 128:boff + (mc + 1) * 128]
                    for nk in range(2):
                        nc.tensor.matmul(
                            ps[:, nk * 512:(nk + 1) * 512],
                            lhsT=lhsT,
                            rhs=gr[:, kc, nk * 512:(nk + 1) * 512],
                            start=(kc == 0), stop=(kc == 1))
                nc.scalar.activation(v_sb[:, mc, :], ps, AF.Identity,
                                     bias=bnv[:, mc:mc + 1])
                nc.gpsimd.dma_start(out=vproj_dram[mc * 128:(mc + 1) * 128, :],
                                    in_=v_sb[:, mc, :])
                # v_aug[j][p2, n, u] = V_att[128j+p2, 64n+u];
                # V_att[s, d] = vproj[s//2, (s%2)*512 + d]
                for j in (2 * mc, 2 * mc + 1):
                    src = vproj_dram[64 * j:64 * j + 64, :]
                    src = src.rearrange("c (h n u) -> c h n u", h=2, n=NH)
                    nc.sync.dma_start(out=v_aug[:, j, :, 0:VS], in_=src)

        def nin_t(p, tps=(0, 1, 2, 3), pool=None):
            """Transposed nin for q/k -> qT_m/kT_m. Pairs (tp, tp+4) share a
            psum tile; one TT moves both with bias."""
            boff = 1280 * BR[p] + 768
            gr = grs[p]
            tgt = qT_m if p == "q" else kT_m
            bslot = 1 if p == "q" else 0
            for tp in tps:
                ps = (pool or psum_main).tile([128, 1024], f32, tag="pm")
                for jj in range(2):
                    hw_p = tp + 4 * jj
                    for kc in range(2):
                        nc.tensor.matmul(
                            ps[:, jj * D:(jj + 1) * D],
                            lhsT=gr[:, kc, hw_p * 128:(hw_p + 1) * 128],
                            rhs=w_all[:, kc, boff:boff + D],
                            start=(kc == 0), stop=(kc == 1))
                # out[p, 2a+jj] = ps[p, jj*512+a] + bn[a]
                out_ap = tgt[:, tp, :].rearrange("p (a jj) -> p a jj", jj=2)
                in_ap = ps.rearrange("p (jj a) -> p a jj", jj=2)
                bn_ap = bnb[:, bslot, :].rearrange("p (a jj) -> p a jj", jj=2)
                eng(cfg["nin_bias_engine"]).tensor_tensor(
                    out_ap, in_ap, bn_ap, Op.add)

        # pipeline: emission order interleaves engines; deps do the rest
        h1_mm("v", eluk)
        h1_mm("k", eluk)
        e1_stage("v")
        h1_mm("q", eluq)
        e1_stage("k")
        h2_mm("v")
        e1_stage("q")
        gr_stage("v", xk)
        h2_mm("k")
        gr_stage("k", xk)
        nin_v()
        h2_mm("q")
        gr_stage("q", xq)
        nin_t("k")

        # ---------------- attention (software-pipelined with nin_q) --------
        ctx_br.close()  # release branch psum banks; nin_q uses psum_att
        with ExitStack() as ctx_a:
            psum_att = ctx_a.enter_context(tc.tile_pool(name="psum_att", bufs=3, space="PSUM"))
            psum_pv = ctx_a.enter_context(tc.tile_pool(name="psum_pv", bufs=2, space="PSUM"))
            eT_pool = ctx_a.enter_context(tc.tile_pool(name="eT", bufs=3))
            att_small = ctx_a.enter_context(tc.tile_pool(name="att_small", bufs=3))

            GROUPS = [(0,), (1, 7), (2, 6), (3, 5), (4,)]
            G = {}
            off = 0
            for grp in GROUPS:
                for j in grp:
                    G[j] = off
                    off += S - 128 * j
            uls = {}

            def scores_head(n):
                tp, po = n // 2, 64 * (n % 2)
                eT = eT_pool.tile([128, 4608], bf16, tag="eT")
                for grp in GROUPS:
                    glen = sum(S - 128 * j for j in grp)
                    gbase = G[grp[0]]
                    ps = psum_att.tile([128, 1024], f32, tag="pm")
                    for j in grp:
                        off = G[j] - gbase
                        lhsT = kT_m[po:po + 64, tp, 128 * j:128 * (j + 1)]
                        for s1a, s1b in _split_psum_ranges(off, off + (S - 128 * j)):
                            nc.tensor.matmul(
                                ps[:, s1a:s1b],
                                lhsT=lhsT,
                                rhs=qT_m[po:po + 64, tp,
                                         128 * j + (s1a - off):128 * j + (s1b - off)],
                                start=True, stop=True)
                    nc.scalar.activation(eT[:, gbase:gbase + glen],
                                         ps[:, 0:glen], AF.Exp, scale=SCALE)
                    for j in grp:
                        eng(cfg["mask_engine"]).tensor_tensor(
                            eT[:, G[j]:G[j] + 128], eT[:, G[j]:G[j] + 128],
                            mask01, Op.mult)
                return eT

            def pv_head(n, eT):
                # PV: j-outer so each v_aug lhsT loads once
                pv0 = psum_pv.tile([VS + 1, 512], f32, tag="pv")
                pv1 = psum_pv.tile([VS + 1, 512], f32, tag="pv")
                for j in range(8):
                    lhsT = v_aug[:, j, n, :]
                    if j <= 3:
                        s1a = max(0, 128 * j)
                        nc.tensor.matmul(
                            pv0[:, s1a:512],
                            lhsT=lhsT,
                            rhs=eT[:, G[j] + (s1a - 128 * j):G[j] + (512 - 128 * j)],
                            start=(j == 0), stop=(j == 3))
                    s1a = max(512, 128 * j)
                    nc.tensor.matmul(
                        pv1[:, s1a - 512:512],
                        lhsT=lhsT,
                        rhs=eT[:, G[j] + (s1a - 128 * j):G[j] + (1024 - 128 * j)],
                        start=(j == 0), stop=(j == 7))
                nc.vector.memset(pv0[VS:VS + 1, 0:1], 1.0)

                ul = att_small.tile([VS + 1, 1024], f32, tag="ul", bufs=4)
                for c, pv in ((0, pv0), (1, pv1)):
                    eng(cfg["ul_engine"]).tensor_copy(
                        ul[:, c * 512:(c + 1) * 512], pv)
                g, b = n // 2, n % 2
                lb = 32 * g + 16 * b
                nc.sync.dma_start(out=lbuf[lb:lb + 16, :],
                                  in_=ul[VS:VS + 1, :])
                uls[n] = ul

                if n % 2 == 0:
                    return
                # head pair (n-1, n) done: 1/l on partitions 32g..32g+31
                nc.vector.reciprocal(rbuf[32 * g:32 * g + 32, :],
                                     lbuf[32 * g:32 * g + 32, :])
                nc.gpsimd.dma_start(out=rbuf_dram[32 * g:32 * g + 32, :],
                                    in_=rbuf[32 * g:32 * g + 32, :])
                rflat = rbuf_dram.rearrange("a b -> (a b)")
                for nn in (n - 1, n):
                    bb_ = nn % 2
                    fin = att_small.tile([VS, 1024], f32, tag="fin", bufs=3)
                    rbc = att_small.tile([VS, 1024], f32, tag="rbc", bufs=4)
                    rd = rflat[(32 * g + 16 * bb_) * 64:
                               (32 * g + 16 * bb_) * 64 + 1024]
                    rsrc = bass.AP(tensor=rd.tensor, offset=rd.offset,
                                   ap=[[0, VS]] + list(rd.ap))
                    nc.gpsimd.dma_start(out=rbc, in_=rsrc)
                    eng(cfg["fin_engine"]).tensor_tensor(
                        fin, uls.pop(nn)[0:VS, :], rbc, Op.mult)
                    nc.sync.dma_start(out=out_d[VS * nn:VS * (nn + 1), :],
                                      in_=fin)

            # emission: ninq(tp) feeds heads 2tp,2tp+1; scores run one head
            # ahead of PV so the in-order PE queue never blocks on eT
            eTs = {}
            nin_t("q", tps=(0,), pool=psum_att)
            eTs[0] = scores_head(0)
            nin_t("q", tps=(1,), pool=psum_att)
            eTs[1] = scores_head(1)
            nin_t("q", tps=(2,), pool=psum_att)
            for n in range(NH):
                if n + 2 < NH:
                    eTs[n + 2] = scores_head(n + 2)
                if n == 0:
                    nin_t("q", tps=(3,), pool=psum_att)
                pv_head(n, eTs.pop(n))

    nc.compile()
    return nc


_CACHE = {}


def _get_program(cfg_key=None):
    key = cfg_key or "default"
    if key not in _CACHE:
        _CACHE[key] = build_program(CFG)
    return _CACHE[key]


def make_in_map(inp, b):
    """Per-core input dict for batch b (weights host-packed/cast/bias-folded)."""
    import ml_dtypes
    wt = np.dtype(ml_dtypes.bfloat16)

    m = {
        "query": np.ascontiguousarray(inp["query"][b].reshape(C, S)).astype(wt),
        "key": np.ascontiguousarray(inp["key"][b].reshape(C, S)).astype(wt),
    }
    wall = np.zeros((C, 3 * 1280), dtype=wt)
    ball = np.zeros((128, 18), dtype=np.float32)
    BR = {"v": 0, "k": 1, "q": 2}
    for p in ("v", "k", "q"):
        src = "v" if p == "v" else p
        w1 = inp[f"{src}_gr_w1"].astype(wt)   # (C, C) row=cout
        w2 = inp[f"{src}_gr_w2"].astype(wt)   # (2C, C)
        wn = inp[f"{src}_nin_w"].astype(wt)   # (D, C)
        boff = 1280 * BR[p]
        wall[:, boff:boff + 256] = w1.T
        wall[:, boff + 256:boff + 768] = w2.T
        wall[:, boff + 768:boff + 1280] = wn.T
        # bias folding for the +1-shifted elu inputs
        b1a = inp[f"{src}_gr_b1"] - w1.astype(np.float32).sum(axis=1)
        b2a = inp[f"{src}_gr_b2"] - w2.astype(np.float32).sum(axis=1)
        b0 = 6 * BR[p]
        ball[:, b0 + 0:b0 + 2] = b1a.reshape(2, 128).T
        ball[:, b0 + 2:b0 + 4] = 0.5 * b2a[0:C].reshape(2, 128).T
        ball[:, b0 + 4:b0 + 6] = 0.5 * b2a[C:2 * C].reshape(2, 128).T
    m["wall"] = wall
    m["ball"] = ball
    m["bnv"] = np.ascontiguousarray(
        inp["v_nin_b"].reshape(4, 128).T).astype(np.float32)
    bnqk = np.zeros((2, 2 * D), dtype=np.float32)
    for slot, p in ((0, "k"), (1, "q")):
        bn = inp[f"{p}_nin_b"].astype(np.float32)
        bnqk[slot, 0::2] = bn
        bnqk[slot, 1::2] = bn
    m["bnqk"] = bnqk.reshape(-1)
    return m


def kernel(**inputs):
    from concourse.bass_utils import run_bass_kernel_spmd

    nc = _get_program()
    inp = {k: np.asarray(v, dtype=np.float32) for k, v in inputs.items()}

    in_maps = [make_in_map(inp, b) for b in range(N_CORES)]

    trace = bool(int(os.environ.get("BASS_KERNEL_TRACE", "0")))
    res = run_bass_kernel_spmd(nc, in_maps, core_ids=list(range(N_CORES)),
                               trace=trace)
    LAST_RUN["exec_time_ns"] = getattr(res, "exec_time_ns", None)
    LAST_RUN["results"] = res
    out = np.stack([res.results[i]["out"].reshape(D, 32, 32)
                    for i in range(N_CORES)])
    return out.astype(np.float32)


LAST_RUN = {}


if __name__ == "__main__":
    nc = build_program()
    print("compiled OK")
